# revision 31
# baseline (speedup 1.0000x reference)
"""Trainium2 Bass kernel for MultiHeadedAttention with learned per-key-position scaling.

Sharding over 8 NeuronCores: batch(2) x q-half(2) x head-half(2).
Each core: its batch's full keys/values, a 1024-row query slice, 6 heads.

Key-position axis is host-permuted per core to [own q-half | other q-half] so
that delta (computed locally from the full query, no collective) lines up
chunk-for-chunk with the key chunks: rdcol is produced in place, no shuffle.

Scores are computed transposed ([kpos, q]) so that:
  - the per-key-position divisor delta folds into the exp's per-partition scale,
  - the softmax denominator Z comes from a ones-column appended to V,
  - P@V needs no on-chip transposition of the attention probabilities.

Attention runs in two key-position rounds (kc 0-7, then 8-15) of all six
heads, with partial (x, Z) spilled to SBUF between rounds. This spreads the
key/value/mask stream DMA over a whole round of Activation work instead of
concentrating it in head 0. Round 1 interleaves heads 0 and 1 per-kc so the
exp stream starts as soon as the first key chunk lands.

All cross-partition broadcasts (1/Z row, biases) are PE outer-products with a
ones vector. The mask multiply runs as scalar_tensor_tensor (DVE 4x mode).

Precision: q/k path in bf16 (inputs, weights and projected tiles); scores
accumulate in f32 PSUM; probabilities, V and output projection in bf16;
softmax normalization in f32.

Host combines per-core partial outputs (sum over head-halves + bo).
"""

import sys

for _p in ("/opt/trn_rl_repo",):
    if _p not in sys.path:
        sys.path.insert(0, _p)

import numpy as np
import ml_dtypes

BF16 = ml_dtypes.bfloat16

B, S, D, H, DK = 2, 2048, 768, 12, 64
NCORES = 8
SQ = S // 2          # query rows per core
HH = H // 2          # heads per core
DH = HH * DK         # 384 head dims per core

_cache = {}


def _build(s=S, sq=SQ, hh=HH, d=D, dk=DK):
    import concourse.bass as bass
    import concourse.mybir as mybir
    import concourse.tile as tile
    from concourse import bacc

    f32 = mybir.dt.float32
    f32r = mybir.dt.float32r
    bf = mybir.dt.bfloat16
    Exp = mybir.ActivationFunctionType.Exp
    mult = mybir.AluOpType.mult
    add = mybir.AluOpType.add
    amin = mybir.AluOpType.min
    amax = mybir.AluOpType.max

    dh = hh * dk
    KC = s // 128        # key-position chunks
    KCH = KC // 2        # chunks per round
    C6 = d // 128        # d_model chunks
    C3 = dh // 128       # output-dim chunks per core
    NQ = sq // 512       # 512-wide q column blocks (attention)
    QC = sq // 128       # q row chunks for output projection
    BW = 256             # streaming block width (projection inputs)
    NBK = s // BW        # key/value stream blocks
    NBQ = s // BW        # full-query stream blocks (delta needs all of q)
    KCL = BW // 128      # kpos chunks per stream block

    nc = bacc.Bacc("TRN2", target_bir_lowering=False, debug=False, num_devices=NCORES)

    t = {}
    t["qT"] = nc.dram_tensor("qT", [d, s], f32r, kind="ExternalInput").ap()
    t["kT"] = nc.dram_tensor("kT", [d, s], f32r, kind="ExternalInput").ap()
    t["vT"] = nc.dram_tensor("vT", [d, s], bf, kind="ExternalInput").ap()
    t["maskT"] = nc.dram_tensor("maskT", [s, sq], bf, kind="ExternalInput").ap()
    t["wqd"] = nc.dram_tensor("wqd", [d, dh + 1], f32r, kind="ExternalInput").ap()
    t["wk"] = nc.dram_tensor("wk", [d, dh], f32r, kind="ExternalInput").ap()
    t["wv"] = nc.dram_tensor("wv", [d, dh], bf, kind="ExternalInput").ap()
    t["wo"] = nc.dram_tensor("wo", [dh, d], bf, kind="ExternalInput").ap()
    t["bqk"] = nc.dram_tensor("bqk", [2 * dh], f32, kind="ExternalInput").ap()
    t["bvd"] = nc.dram_tensor("bvd", [dh + 2 + 128], f32r, kind="ExternalInput").ap()
    t["yp"] = nc.dram_tensor("yp", [sq, d], bf, kind="ExternalOutput").ap()

    # [d, *] tensors viewed as [128, C6, *] partition tiles
    def dview(ap):
        return ap.rearrange("(c p) s -> p c s", p=128)

    def row1(ap):
        # view a 1-D DRAM vector as a single-partition [1, n] transfer
        return bass.AP(tensor=ap.tensor, offset=ap.offset, ap=[[0, 1]] + list(ap.ap))

    with tile.TileContext(nc) as tc:
        with (
            tc.tile_pool(name="persist", bufs=1) as P,
            tc.tile_pool(name="pj", bufs=2, space="PSUM") as PJ,
            tc.tile_pool(name="xpp", bufs=2, space="PSUM") as XPP,
            tc.tile_pool(name="work", bufs=5) as W,
            tc.tile_pool(name="work2", bufs=2) as W2,
            tc.tile_pool(name="load", bufs=4) as L,
            tc.tile_pool(name="loadfr", bufs=2) as LF,
        ):
            maskT = P.tile([128, KC, sq], bf)
            vsb = P.tile([128, KC, hh, dk + 1], bf)
            qTh = P.tile([128, C3, sq], f32r)    # head pairs packed on partitions
            kTh = P.tile([128, C3, s], f32r)
            xT = P.tile([128, C3, sq], bf)
            xsp = P.tile([128, hh, sq], bf)      # round-1 partial (x, Z) spill
            wqd_sb = P.tile([128, C6, dh + 1], f32r)
            wk_sb = P.tile([128, C6, dh], f32r)
            wv_sb = P.tile([128, C6, dh], bf)
            wo_sb = P.tile([128, C3, d], bf)
            bqkc = P.tile([128, 2 * C3], f32)
            bvb = P.tile([128, hh, dk], f32)
            bdb = P.tile([128, 1], f32)
            bvdrow = P.tile([1, dh + 2 + 128], f32r)
            rdcol = P.tile([128, KC], f32)

            # warm the ACT exp table while DMAs stream
            dummy = W.tile([1, 2], f32, tag="dummy")
            nc.vector.memset(dummy, 0.0)
            nc.scalar.activation(dummy, dummy, Exp, scale=1.0)

            nc.sync.dma_start(wqd_sb, dview(t["wqd"]))
            nc.sync.dma_start(bvdrow, row1(t["bvd"]))
            nc.sync.dma_start(bqkc, t["bqk"].rearrange("(c p) -> p c", p=128))
            wd_sb = wqd_sb[:, :, dh : dh + 1]
            wq_sb = wqd_sb[:, :, 0:dh]
            bqc = bqkc[:, 0:C3]
            bkc = bqkc[:, C3 : 2 * C3]
            bdrow = bvdrow[:, dh : dh + 2]
            onesr = bvdrow[:, dh + 2 : dh + 2 + 128]
            bvrow = bvdrow[:, 0:dh]
            nc.vector.memset(vsb[:, :, :, dk : dk + 1], 1.0)

            # bd broadcast across partitions via PE outer-product
            bdp = PJ.tile([128, 2], f32, tag="pj")
            nc.tensor.matmul(bdp, lhsT=onesr, rhs=bdrow, start=True, stop=True)
            nc.vector.tensor_copy(bdb, bdp[:, 0:1])

            def bf_block(src_ap, blk, dt=bf):
                fr = L.tile([128, C6, BW], dt, tag="ldf")
                nc.sync.dma_start(fr, src_ap[:, :, blk * BW : (blk + 1) * BW])
                return fr

            # --- full-query stream: delta everywhere, Q projection own half
            def q_block(blk):
                qqb = bf_block(dview(t["qT"]), blk, f32r)
                if blk == 1:
                    nc.sync.dma_start(wk_sb, dview(t["wk"]))
                # delta for this block's kpos chunks, finished in place
                dps = PJ.tile([128, KCL], f32, tag="pj")
                qqf = qqb.bitcast(f32)
                wdf = wd_sb.bitcast(f32)
                for kcl in range(KCL):
                    for c in range(C6):
                        nc.tensor.matmul(
                            dps[:, kcl : kcl + 1],
                            lhsT=qqf[:, c, kcl * 128 : (kcl + 1) * 128],
                            rhs=wdf[:, c, :],
                            start=(c == 0),
                            stop=(c == C6 - 1),
                        )
                dl = W2.tile([128, KCL], f32, tag="dl")
                nc.vector.tensor_scalar(
                    out=dl, in0=dps, scalar1=bdb, scalar2=0.0, op0=add, op1=amax
                )
                nc.vector.tensor_scalar(
                    out=dl, in0=dl, scalar1=8.0, scalar2=1.0, op0=amin, op1=add
                )
                nc.vector.reciprocal(rdcol[:, blk * KCL : (blk + 1) * KCL], dl)
                # Q projection only over own half (first NBQ/2 blocks)
                if blk < NBQ // 2:
                    for m in range(C3):
                        qp = PJ.tile([128, BW], f32, tag="pj")
                        for c in range(C6):
                            nc.tensor.matmul(
                                qp,
                                lhsT=wq_sb[:, c, m * 128 : (m + 1) * 128],
                                rhs=qqb[:, c, :],
                                start=(c == 0),
                                stop=(c == C6 - 1),
                            )
                        nc.vector.tensor_scalar_add(
                            out=qTh[:, m, blk * BW : (blk + 1) * BW],
                            in0=qp,
                            scalar1=bqc[:, m : m + 1],
                        )

            kv_tiles = {}

            def k_dma(blk):
                kv_tiles[("k", blk)] = bf_block(dview(t["kT"]), blk, f32r)

            def k_mm(blk, m):
                kfb = kv_tiles[("k", blk)]
                kp = PJ.tile([128, BW], f32, tag="pj")
                for c in range(C6):
                    nc.tensor.matmul(
                        kp,
                        lhsT=wk_sb[:, c, m * 128 : (m + 1) * 128],
                        rhs=kfb[:, c, :],
                        start=(c == 0),
                        stop=(c == C6 - 1),
                    )
                nc.vector.tensor_scalar_add(
                    out=kTh[:, m, blk * BW : (blk + 1) * BW],
                    in0=kp,
                    scalar1=bkc[:, m : m + 1],
                )

            def k_block(blk):
                k_dma(blk)
                for m in range(C3):
                    k_mm(blk, m)

            def v_dma(blk):
                vT = LF.tile([128, C6, BW], bf, tag="vb")
                nc.sync.dma_start(
                    vT, dview(t["vT"])[:, :, blk * BW : (blk + 1) * BW]
                )
                kv_tiles[("v", blk)] = vT

            def v_mm(blk, kcl):
                vT = kv_tiles[("v", blk)]
                kc = blk * KCL + kcl
                vp = PJ.tile([128, dh], f32, tag="pj")
                for c in range(C6):
                    nc.tensor.matmul(
                        vp,
                        lhsT=vT[:, c, kcl * 128 : (kcl + 1) * 128],
                        rhs=wv_sb[:, c, :],
                        start=(c == 0),
                        stop=(c == C6 - 1),
                    )
                nc.vector.tensor_tensor(
                    out=vsb[:, kc, :, 0:dk],
                    in0=vp.rearrange("p (h e) -> p h e", h=hh),
                    in1=bvb,
                    op=add,
                )

            def v_block(blk):
                v_dma(blk)
                for kcl in range(KCL):
                    v_mm(blk, kcl)

            def mask_kc(kc):
                nc.sync.dma_start(
                    maskT[:, kc : kc + 1, :],
                    t["maskT"].rearrange("(kc p) q -> p kc q", p=128)[
                        :, kc : kc + 1, :
                    ],
                )

            for blk in range(NBQ // 2):
                q_block(blk)
            k_block(0)
            mask_kc(0)
            nc.sync.dma_start(wv_sb, dview(t["wv"]))
            bvp = PJ.tile([128, dh], f32, tag="pj")
            nc.tensor.matmul(bvp, lhsT=onesr, rhs=bvrow, start=True, stop=True)
            nc.vector.tensor_copy(bvb.rearrange("p h e -> p (h e)"), bvp)
            v_block(0)

            # attention step split for software pipelining: part A
            # (scores+exp+mask) of step i+1 is emitted before part B (PV) of
            # step i, so the PE never sits behind the exp/mask chain.
            def attn_A(h, kc):
                hoff = (h % 2) * 64
                sps = PJ.tile([128, sq], f32, tag="pj")
                for nn in range(NQ):
                    nc.tensor.matmul(
                        sps[:, nn * 512 : (nn + 1) * 512],
                        lhsT=kTh[hoff : hoff + 64, h // 2, kc * 128 : (kc + 1) * 128],
                        rhs=qTh[hoff : hoff + 64, h // 2, nn * 512 : (nn + 1) * 512],
                        start=True,
                        stop=True,
                    )
                psb = W.tile([128, sq], bf, tag="psb")
                if h == 0 and kc == 0:
                    # split first exp so the ACT stream starts earlier
                    for nn in range(NQ):
                        nc.scalar.activation(
                            psb[:, nn * 512 : (nn + 1) * 512],
                            sps[:, nn * 512 : (nn + 1) * 512],
                            Exp,
                            scale=rdcol[:, kc : kc + 1],
                        )
                else:
                    nc.scalar.activation(psb, sps, Exp, scale=rdcol[:, kc : kc + 1])
                nc.vector.tensor_tensor(
                    out=psb, in0=psb, in1=maskT[:, kc, :], op=mult
                )
                return psb

            def attn_B(st):
                h, kci, xps, psb = st["h"], st["kci"], st["xps"], st["psb"]
                for nn in range(NQ):
                    nc.tensor.matmul(
                        xps[:, nn * 512 : (nn + 1) * 512],
                        lhsT=vsb[:, st["kc"], h, :],
                        rhs=psb[:, nn * 512 : (nn + 1) * 512],
                        start=(kci == 0),
                        stop=(kci == KCH - 1),
                    )

            def spill(h, xps):
                nc.vector.tensor_copy(xsp[0 : dk + 1, h, :], xps[0 : dk + 1, :])

            # finalize for head h, split into pieces so the DVE/PE chain and
            # its PSUM-ring slot use interleave with the next head's steps
            def fin_make(h, xps):
                hoff = (h % 2) * 64
                st = {}

                def p_add():
                    st["xsf"] = W2.tile([dk + 1, sq], f32, tag="xsf", name=f"xsf{h}")
                    nc.vector.tensor_tensor(
                        out=st["xsf"], in0=xps[0 : dk + 1, :],
                        in1=xsp[0 : dk + 1, h, :], op=add,
                    )

                def p_recip():
                    st["rz"] = W2.tile([1, sq], f32r, tag="rz", name=f"rz{h}")
                    with nc.allow_low_precision(reason="f32r 1/Z for PE broadcast"):
                        nc.vector.reciprocal(st["rz"], st["xsf"][dk : dk + 1, :])

                def p_outer():
                    st["rzp"] = PJ.tile([64, sq], f32, tag="pj", name=f"rzp{h}")
                    rzr = st["rz"]
                    for nn in range(NQ):
                        nc.tensor.matmul(
                            st["rzp"][:, nn * 512 : (nn + 1) * 512],
                            lhsT=onesr[0:1, 0:64],
                            rhs=rzr[:, nn * 512 : (nn + 1) * 512],
                            start=True,
                            stop=True,
                        )

                def p_mult():
                    nc.vector.tensor_tensor(
                        out=xT[hoff : hoff + 64, h // 2, :],
                        in0=st["xsf"][0:dk, :],
                        in1=st["rzp"],
                        op=mult,
                    )

                return [p_add, p_recip, p_outer, p_mult]

            # emission schedules: streams paced behind the attention steps,
            # split into per-piece items so no single kci gets a 2us PE burst
            em_pair = {
                0: [("kd", 1), ("km", 1, 0), ("m", 1), ("m", 2)],
                1: [("km", 1, 1), ("km", 1, 2), ("vd", 1)],
                2: [("vm", 1, 0), ("vm", 1, 1), ("kd", 2), ("m", 3)],
                3: [("km", 2, 0), ("km", 2, 1), ("m", 4)],
                4: [("km", 2, 2), ("vd", 2), ("m", 5)],
                5: [("vm", 2, 0), ("vm", 2, 1), ("kd", 3), ("km", 3, 0), ("m", 6)],
                6: [("km", 3, 1), ("km", 3, 2), ("m", 7), ("vd", 3)],
                7: [("vm", 3, 0), ("vm", 3, 1)],
            }
            em_h = {
                2: {0: [("m", 7), ("kd", 4)], 1: [("km", 4, 0)], 2: [("km", 4, 1)],
                    3: [("km", 4, 2)], 4: [("vd", 4), ("vm", 4, 0)],
                    5: [("vm", 4, 1)], 6: [("m", 8)], 7: [("m", 9)]},
                3: {0: [("kd", 5), ("km", 5, 0)], 1: [("km", 5, 1)], 2: [("km", 5, 2)],
                    3: [("vd", 5), ("vm", 5, 0)], 4: [("vm", 5, 1)],
                    5: [("m", 10)], 6: [("m", 11)], 7: [("q", 4)]},
                4: {0: [("kd", 6), ("km", 6, 0)], 1: [("km", 6, 1)], 2: [("km", 6, 2)],
                    3: [("vd", 6), ("vm", 6, 0)], 4: [("vm", 6, 1)],
                    5: [("q", 5)], 6: [("q", 6)], 7: [("wo", 0)]},
                5: {0: [("q", 7)], 1: [("m", 12)], 2: [("m", 13)]},
            }
            em_r2 = {
                0: [("kd", 7), ("km", 7, 0)], 1: [("km", 7, 1)], 2: [("km", 7, 2)],
                3: [("vd", 7), ("vm", 7, 0)], 4: [("vm", 7, 1), ("m", 14)],
                5: [("m", 15)],
            }

            def emit(lst):
                for item in lst:
                    kind, i = item[0], item[1]
                    if kind == "kd":
                        k_dma(i)
                    elif kind == "km":
                        k_mm(i, item[2])
                    elif kind == "vd":
                        v_dma(i)
                    elif kind == "vm":
                        v_mm(i, item[2])
                    elif kind == "m":
                        mask_kc(i)
                    elif kind == "q":
                        q_block(i)
                    elif kind == "wo":
                        nc.sync.dma_start(
                            wo_sb, t["wo"].rearrange("(c p) m -> p c m", p=128)
                        )

            FIN_AT = {1: 0, 2: 1, 4: 2, 5: 3}
            fin_state = {"p": None}

            def gen_steps():
                # round 1: pair (h0, h1) interleaved per kci
                xps0 = XPP.tile([dk + 1, sq], f32, tag="xps", name="xps_r1h0")
                xps1 = XPP.tile([dk + 1, sq], f32, tag="xps", name="xps_r1h1")
                for kci in range(KCH):
                    postB0 = [] if kci < KCH - 1 else [lambda: spill(0, xps0)]
                    yield dict(h=0, kc=kci, kci=kci, xps=xps0, postA=[], postB=postB0)
                    postA1 = [lambda k=kci: emit(em_pair[k])]
                    postB1 = [] if kci < KCH - 1 else [lambda: spill(1, xps1)]
                    yield dict(h=1, kc=kci, kci=kci, xps=xps1, postA=postA1, postB=postB1)
                # round 1 singles
                for h in range(2, hh):
                    xps = XPP.tile([dk + 1, sq], f32, tag="xps", name=f"xps_r1h{h}")
                    for kci in range(KCH):
                        postA = [lambda h=h, k=kci: emit(em_h[h].get(k, []))]
                        postB = [] if kci < KCH - 1 else [lambda h=h, x=xps: spill(h, x)]
                        yield dict(h=h, kc=kci, kci=kci, xps=xps, postA=postA, postB=postB)
                # round 2, finalize(h-1) pieces as postB hooks
                for h in range(hh):
                    xps = XPP.tile([dk + 1, sq], f32, tag="xps", name=f"xps_r2h{h}")
                    for kci in range(KCH):
                        postA = []
                        postB = []
                        if h == 0 and kci in em_r2:
                            postA.append(lambda k=kci: emit(em_r2[k]))
                        if h > 0 and kci in FIN_AT:
                            postB.append(lambda i=FIN_AT[kci]: fin_state["p"][i]())
                        if kci == KCH - 1:
                            postB.append(
                                lambda h=h, x=xps: fin_state.__setitem__(
                                    "p", fin_make(h, x)
                                )
                            )
                        yield dict(h=h, kc=KCH + kci, kci=kci, xps=xps,
                                   postA=postA, postB=postB)

            from collections import deque

            LAG = 3
            q = deque()
            for st in gen_steps():
                st["psb"] = attn_A(st["h"], st["kc"])
                for f in st["postA"]:
                    f()
                if len(q) >= LAG:
                    done = q.popleft()
                    attn_B(done)
                    for f in done["postB"]:
                        f()
                q.append(st)
            while q:
                done = q.popleft()
                attn_B(done)
                for f in done["postB"]:
                    f()
            pf = fin_state["p"]
            pf[0]()
            pf[1]()
            pf[2]()

            ysbs = {}

            def oproj_c01(qcs):
                for qc in qcs:
                    yps = (PJ if qc % 2 == 0 else XPP).tile(
                        [128, d], f32, tag="pj" if qc % 2 == 0 else "xps",
                        name=f"yps{qc}",
                    )
                    ysbs[qc] = yps
                    for c in range(2):
                        for col in range(0, d, 512):
                            ncol = min(512, d - col)
                            nc.tensor.matmul(
                                yps[:, col : col + ncol],
                                lhsT=xT[:, c, qc * 128 : (qc + 1) * 128],
                                rhs=wo_sb[:, c, col : col + ncol],
                                start=(c == 0),
                                stop=False,
                            )

            def oproj_c2(qcs):
                for qc in qcs:
                    yps = ysbs[qc]
                    for col in range(0, d, 512):
                        ncol = min(512, d - col)
                        nc.tensor.matmul(
                            yps[:, col : col + ncol],
                            lhsT=xT[:, 2, qc * 128 : (qc + 1) * 128],
                            rhs=wo_sb[:, 2, col : col + ncol],
                            start=False,
                            stop=True,
                        )
                    ysb = W2.tile([128, d], bf, tag="ysb", bufs=QC, name=f"ysb{qc}")
                    if qc % 2 == 0:
                        nc.scalar.copy(ysb, yps)
                    else:
                        nc.vector.tensor_copy(ysb, yps)
                    nc.sync.dma_start(t["yp"][qc * 128 : (qc + 1) * 128, :], ysb)

            oproj_c01([0, 1, 2])
            pf[3]()
            oproj_c2([0, 1, 2])
            oproj_c01([3, 4, 5])
            oproj_c2([3, 4, 5])
            oproj_c01([6, 7])
            oproj_c2([6, 7])

    nc.compile()
    return nc


def _in_maps(query, key, value, mask, Wq, bq, Wk, bk, Wv, bv, Wo, Wd, bd, sq=SQ, dh=DH):
    query = np.asarray(query, np.float32)
    key = np.asarray(key, np.float32)
    value = np.asarray(value, np.float32)
    mask = np.asarray(mask)
    qT = [np.ascontiguousarray(query[b].T) for b in range(B)]
    kT = [np.ascontiguousarray(key[b].T) for b in range(B)]
    vT = [np.ascontiguousarray(value[b].T).astype(BF16) for b in range(B)]
    wqb = np.ascontiguousarray(Wq, np.float32)
    wkb = np.ascontiguousarray(Wk, np.float32)
    wvb = np.ascontiguousarray(Wv).astype(BF16)
    wob = np.ascontiguousarray(Wo).astype(BF16)
    wdb = np.ascontiguousarray(Wd, np.float32)
    bqf = np.ascontiguousarray(bq, np.float32)
    bkf = np.ascontiguousarray(bk, np.float32)
    bvf = np.ascontiguousarray(bv, np.float32)
    bdf = np.ascontiguousarray(bd, np.float32)

    maps = []
    for c in range(NCORES):
        b, qh, hf = c // 4, (c // 2) % 2, c % 2
        qs = slice(qh * sq, (qh + 1) * sq)
        qo = slice((1 - qh) * sq, (2 - qh) * sq)  # other q-half
        hs = slice(hf * dh, (hf + 1) * dh)
        # key-position axis permuted to [own q-half | other q-half]
        perm = np.r_[qh * sq : (qh + 1) * sq, (1 - qh) * sq : (2 - qh) * sq]
        maps.append(
            {
                "qT": np.ascontiguousarray(
                    np.concatenate([qT[b][:, qs], qT[b][:, qo]], axis=1)
                ),
                "kT": np.ascontiguousarray(kT[b][:, perm]),
                "vT": np.ascontiguousarray(vT[b][:, perm]),
                "maskT": np.ascontiguousarray(mask[b, qs].T[perm]).astype(BF16),
                "wqd": np.ascontiguousarray(
                    np.concatenate([wqb[:, hs], wdb], axis=1)
                ),
                "wk": np.ascontiguousarray(wkb[:, hs]),
                "wv": np.ascontiguousarray(wvb[:, hs]),
                "wo": np.ascontiguousarray(wob[hs, :]),
                "bqk": np.ascontiguousarray(
                    np.concatenate([bqf[hs], bkf[hs]])
                ),
                "bvd": np.ascontiguousarray(
                    np.concatenate([bvf[hs], bdf, bdf, np.ones(128, np.float32)])
                ),
            }
        )
    return maps


def kernel(query, key, value, mask, Wq, bq, Wk, bk, Wv, bv, Wo, bo, Wd, bd):
    from concourse.bass_utils import run_bass_kernel_spmd

    if "nc" not in _cache:
        _cache["nc"] = _build()
    nc = _cache["nc"]

    maps = _in_maps(query, key, value, mask, Wq, bq, Wk, bk, Wv, bv, Wo, Wd, bd)
    res = run_bass_kernel_spmd(nc, maps, core_ids=list(range(NCORES)))

    bof = np.asarray(bo, np.float32)
    y = np.empty((B, S, D), np.float32)
    for b in range(B):
        for qh in range(2):
            c0 = b * 4 + qh * 2
            y[b, qh * SQ : (qh + 1) * SQ] = (
                res.results[c0]["yp"].astype(np.float32)
                + res.results[c0 + 1]["yp"].astype(np.float32)
                + bof[None, :]
            )
    return y
